# revision 5
# baseline (speedup 1.0000x reference)
"""Trainium2 Bass kernel for the DemandAwareRS forward pass, SPMD over 8 NeuronCores.

Math notes (exact algebraic simplifications of the reference, fp32 throughout):
  * demand_score (B,ND,S) is computed by the reference but unused in its outputs.
  * P_v[b,v] = sum_n (h[b]@emb[v]) * softmax_n(...)[n,v] = h[b]@emb[v], because the
    per-candidate demand distribution is a softmax over the ND axis (sums to 1) and
    h/graph_rep/last are identical across the ND axis (gnn is a broadcast).
  * infomax_loss = mean(softplus(-pos)) + mean(softplus(neg)) over (B,L);
    the ND axis is an exact duplicate.

Sharding:
  * vocab (n_items) dim of emb_i sharded 8 ways for the bnv candidate-score einsum;
  * the (B,L,D) node-embedding gather and session stats are batch-sharded (16
    sessions/core, 800 rows each via one-descriptor-per-partition indirect DMAs);
    per-batch reductions use host-built one-hot selector matmuls so the SPMD
    program is identical across cores;
  * a 64KB AllReduce assembles [graph_sum | last] (128,128) on every core and a
    4-byte AllReduce combines the infomax-loss partials.
"""

import numpy as np

import concourse.bass as bass
import concourse.mybir as mybir
import concourse.tile as tile
from concourse import bacc
from concourse.bass import IndirectOffsetOnAxis
from concourse.bass_utils import run_bass_kernel_spmd

# problem dims (hardcoded per harness contract)
B, L, S, V, C = 128, 50, 100, 100000, 1000
D, DC, H, ND = 64, 64, 64, 4
N_CORES = 8
BS = B // N_CORES          # 16 sessions per core
NROW = BS * L              # 800 gathered rows per core
NT = (NROW + 127) // 128   # 7 row tiles (last padded to 128)
VS = V // N_CORES          # 12500 vocab rows per core
HV = VS // 2               # 6250  (two vocab halves packed on 128 partitions)
UT = 512                   # u-tile width (PSUM bank = 512 fp32)

f32 = mybir.dt.float32
i32 = mybir.dt.int32
AF = mybir.ActivationFunctionType
AX = mybir.AxisListType

# f32 blob column layout
_ID0 = 0          # identity (128,128)
_W2LO = 128       # [W_pvsd | 0]  (128,128)
_W2HI = 256       # [0 | W_pvsd]  (128,128)
_BLO = 384        # [b_pvsd; 0]   (128,1)
_BHI = 385        # [0; b_pvsd]   (128,1)
_WIMT = 386       # W_im.T        (64,64) on partitions 0:64
_MASK = 450       # mask as f32   (128,50)
_ONES = 500       # ones          (128,1)
_TMASK = 501      # tail-tile row validity mask (128,1)
_FBLOB_W = 502

_STATE = {}


def _build_nc():
    nc = bacc.Bacc(None, num_devices=N_CORES)

    fblob = nc.declare_dram_parameter("fblob", [128, _FBLOB_W], f32, isOutput=False)
    gidx = nc.declare_dram_parameter("gidx", [128, NT], i32, isOutput=False)
    emb = nc.declare_dram_parameter("emb", [V, D], f32, isOutput=False)
    embt = nc.declare_dram_parameter("embt", [2 * D, HV], f32, isOutput=False)
    selg = nc.declare_dram_parameter("selg", [128, NT * 128], f32, isOutput=False)
    selt = nc.declare_dram_parameter("selt", [128, NT * 128], f32, isOutput=False)
    sell = nc.declare_dram_parameter("sell", [128, NT * 128], f32, isOutput=False)
    pv = nc.declare_dram_parameter("pv", [B, VS], f32, isOutput=True)
    se_out = nc.declare_dram_parameter("se_out", [NROW, D], f32, isOutput=True)
    loss = nc.declare_dram_parameter("loss", [1, 1], f32, isOutput=True)

    # collective bounce buffers (internal DRAM)
    sr_in = nc.dram_tensor("sr_in", [128, 128], f32)
    sr_gout = nc.dram_tensor("sr_gout", [128, 128], f32, addr_space="Shared")
    ls_in = nc.dram_tensor("ls_in", [1, 1], f32)
    ls_gout = nc.dram_tensor("ls_gout", [1, 1], f32, addr_space="Shared")

    groups = [list(range(N_CORES))]

    with tile.TileContext(nc) as tc:
        with (
            tc.tile_pool(name="persist", bufs=1) as pp,
            tc.tile_pool(name="stream", bufs=4) as pstream,
            tc.tile_pool(name="ostage", bufs=4) as postage,
            tc.tile_pool(name="psb", bufs=3, space="PSUM") as ppsum_big,
            tc.tile_pool(name="pst", bufs=2, space="PSUM") as ppsum_tiny,
        ):
            # ---- constant / index loads ----
            fb = pp.tile([128, _FBLOB_W], f32, tag="fblob")
            nc.sync.dma_start(fb[:], fblob[:])
            gi = pp.tile([128, NT], i32, tag="gidx")
            nc.sync.dma_start(gi[:], gidx[:])
            sg = pp.tile([128, NT * 128], f32, tag="selg")
            nc.sync.dma_start(sg[:], selg[:])
            st = pp.tile([128, NT * 128], f32, tag="selt")
            nc.sync.dma_start(st[:], selt[:])
            sl = pp.tile([128, NT * 128], f32, tag="sell")
            nc.sync.dma_start(sl[:], sell[:])

            identity = fb[:, _ID0 : _ID0 + 128]
            w2lo = fb[:, _W2LO : _W2LO + 128]
            w2hi = fb[:, _W2HI : _W2HI + 128]
            blo = fb[:, _BLO : _BLO + 1]
            bhi = fb[:, _BHI : _BHI + 1]
            wimt = fb[0:64, _WIMT : _WIMT + 64]
            maskf = fb[:, _MASK : _MASK + L]
            ones = fb[:, _ONES : _ONES + 1]
            tmask = fb[:, _TMASK : _TMASK + 1]

            # ---- gather this core's 800 session-embedding rows (local flat row
            # r = b_local*L + l lands at tile r//128, partition r%128) ----
            rows = []
            for k in range(NT):
                t = pp.tile([128, D], f32, tag=f"rows{k}")
                nc.gpsimd.indirect_dma_start(
                    out=t[:],
                    out_offset=None,
                    in_=emb[:, :],
                    in_offset=IndirectOffsetOnAxis(ap=gi[:, k : k + 1], axis=0),
                )
                rows.append(t)
            # gnn output slice for this core (host broadcasts over ND)
            for k in range(NT):
                nrows = min(128, NROW - k * 128)
                nc.sync.dma_start(
                    se_out[k * 128 : k * 128 + nrows, :], rows[k][0:nrows, :]
                )

            # ---- per-batch stats via one-hot selector matmuls ----
            # sr_partial[b, 0:64]  = sum_l emb-row of (b,l)  (global b, zero off-core)
            # sr_partial[b, 64:128] = last-clicked row of b
            srp_ps = ppsum_tiny.tile([128, 128], f32, tag="tiny")
            for k in range(NT):
                nc.tensor.matmul(
                    out=srp_ps[:, 0:64],
                    lhsT=sg[:, k * 128 : (k + 1) * 128],
                    rhs=rows[k][:],
                    start=(k == 0),
                    stop=(k == NT - 1),
                )
            for k in range(NT):
                nc.tensor.matmul(
                    out=srp_ps[:, 64:128],
                    lhsT=sl[:, k * 128 : (k + 1) * 128],
                    rhs=rows[k][:],
                    start=(k == 0),
                    stop=(k == NT - 1),
                )
            srp = pp.tile([128, 128], f32, tag="srp")
            nc.vector.tensor_copy(srp[:], srp_ps[:])
            nc.sync.dma_start(sr_in[:], srp[:])
            nc.gpsimd.collective_compute(
                "AllReduce",
                mybir.AluOpType.add,
                replica_groups=groups,
                ins=[sr_in[:]],
                outs=[sr_gout[:]],
            )
            srr = pp.tile([128, 128], f32, tag="srr")
            nc.sync.dma_start(srr[:], sr_gout[:])

            # sr = [graph_rep | last]; graph_rep = graph_sum / mask_sum
            msum = pp.tile([B, 1], f32, tag="msum")
            nc.vector.reduce_sum(out=msum[:], in_=maskf, axis=AX.X)
            recip = pp.tile([B, 1], f32, tag="recip")
            nc.vector.reciprocal(recip[:], msum[:])
            sr = pp.tile([128, 128], f32, tag="sr")
            nc.vector.tensor_scalar_mul(sr[:, 0:64], srr[:, 0:64], recip[:])
            nc.vector.tensor_copy(sr[:, 64:128], srr[:, 64:128])

            # srT = sr.T via PE transpose
            srt_ps = ppsum_tiny.tile([128, 128], f32, tag="tiny")
            nc.tensor.transpose(out=srt_ps[:], in_=sr[:], identity=identity)
            srt = pp.tile([128, 128], f32, tag="srt")
            nc.vector.tensor_copy(srt[:], srt_ps[:])

            # hT packed for the two vocab halves:
            #   hlo = [tanh(W_pvsd.T @ srT + b) ; 0], hhi = [0 ; tanh(...)]
            hlo_ps = ppsum_tiny.tile([128, 128], f32, tag="tiny")
            nc.tensor.matmul(out=hlo_ps[:], lhsT=w2lo, rhs=srt[:], start=True, stop=True)
            hlo = pp.tile([128, 128], f32, tag="hlo")
            nc.scalar.activation(out=hlo[:], in_=hlo_ps[:], func=AF.Tanh, bias=blo)
            hhi_ps = ppsum_tiny.tile([128, 128], f32, tag="tiny")
            nc.tensor.matmul(out=hhi_ps[:], lhsT=w2hi, rhs=srt[:], start=True, stop=True)
            hhi = pp.tile([128, 128], f32, tag="hhi")
            nc.scalar.activation(out=hhi[:], in_=hhi_ps[:], func=AF.Tanh, bias=bhi)

            # Q^T = (W_im.T).T @ graph_rep.T -> (d, b); roll along b (free dim)
            qt_ps = ppsum_tiny.tile([64, 128], f32, tag="tiny")
            nc.tensor.matmul(
                out=qt_ps[:], lhsT=wimt, rhs=srt[0:64, :], start=True, stop=True
            )
            qt = pp.tile([64, 128], f32, tag="qt")
            nc.vector.tensor_copy(qt[:], qt_ps[:])
            qrt = pp.tile([64, 128], f32, tag="qrt")
            nc.vector.tensor_copy(qrt[:, 1:128], qt[:, 0:127])
            nc.vector.tensor_copy(qrt[:, 0:1], qt[:, 127:128])

            # back to (b, d) layout
            q_ps = ppsum_tiny.tile([128, 64], f32, tag="tiny")
            nc.tensor.transpose(out=q_ps[:], in_=qt[:], identity=identity[0:64, 0:64])
            q_sb = pp.tile([128, 64], f32, tag="q")
            nc.vector.tensor_copy(q_sb[:], q_ps[:])
            qr_ps = ppsum_tiny.tile([128, 64], f32, tag="tiny")
            nc.tensor.transpose(out=qr_ps[:], in_=qrt[:], identity=identity[0:64, 0:64])
            qr_sb = pp.tile([128, 64], f32, tag="qr")
            nc.vector.tensor_copy(qr_sb[:], qr_ps[:])

            # ---- infomax loss partial over this core's rows:
            # softplus(-pos) + softplus(neg); Qrep_k[r] = Q[b(r)] via selectors ----
            acc = pp.tile([128, 1], f32, tag="acc")
            nc.vector.memset(acc[:], 0.0)
            for name, qq in (("pos", q_sb), ("neg", qr_sb)):
                scale = -1.0 if name == "pos" else 1.0
                for k in range(NT):
                    qrep_ps = ppsum_tiny.tile([128, 64], f32, tag="tiny")
                    nc.tensor.matmul(
                        out=qrep_ps[:],
                        lhsT=st[:, k * 128 : (k + 1) * 128],
                        rhs=qq[:],
                        start=True,
                        stop=True,
                    )
                    prod = pp.tile([128, D], f32, tag=f"prod_{name}{k}")
                    nc.vector.tensor_mul(prod[:], rows[k][:], qrep_ps[:])
                    red = pp.tile([128, 1], f32, tag=f"red_{name}{k}")
                    nc.vector.reduce_sum(out=red[:], in_=prod[:], axis=AX.X)
                    # softplus(scale*x) = ln(1 + exp(scale*x))
                    ex = pp.tile([128, 1], f32, tag=f"ex_{name}{k}")
                    nc.scalar.activation(out=ex[:], in_=red[:], func=AF.Exp, scale=scale)
                    sp = pp.tile([128, 1], f32, tag=f"sp_{name}{k}")
                    nc.scalar.activation(out=sp[:], in_=ex[:], func=AF.Ln, bias=1.0)
                    if k == NT - 1 and NROW % 128:
                        nc.vector.tensor_mul(sp[:], sp[:], tmask)
                    nc.vector.tensor_add(acc[:], acc[:], sp[:])

            loss_ps = ppsum_tiny.tile([1, 1], f32, tag="tiny")
            nc.tensor.matmul(out=loss_ps[:], lhsT=ones, rhs=acc[:], start=True, stop=True)
            loss_sb = pp.tile([1, 1], f32, tag="loss")
            nc.vector.tensor_scalar_mul(loss_sb[:], loss_ps[:], 1.0 / float(B * L))
            nc.sync.dma_start(ls_in[:], loss_sb[:])
            nc.gpsimd.collective_compute(
                "AllReduce",
                mybir.AluOpType.add,
                replica_groups=groups,
                ins=[ls_in[:]],
                outs=[ls_gout[:]],
            )
            nc.gpsimd.dma_start(loss[:], ls_gout[:])

            # ---- P_v = h @ emb_shard.T, streamed over packed vocab tiles ----
            n_full, rem = divmod(HV, UT)
            widths = [UT] * n_full + ([rem] if rem else [])
            off = 0
            for w in widths:
                t = pstream.tile([128, w], f32, tag="embt")
                nc.sync.dma_start(t[:], embt[:, off : off + w])
                for half, hT in ((0, hlo), (1, hhi)):
                    ps = ppsum_big.tile([128, w], f32, tag="pvps")
                    nc.tensor.matmul(
                        out=ps[:], lhsT=hT[:], rhs=t[:], start=True, stop=True
                    )
                    o = postage.tile([128, w], f32, tag="pvout")
                    nc.vector.tensor_copy(o[:], ps[:])
                    col = half * HV + off
                    nc.scalar.dma_start(pv[:, col : col + w], o[:])
                off += w

    nc.compile()
    return nc


def _get_nc():
    if "nc" not in _STATE:
        _STATE["nc"] = _build_nc()
    return _STATE["nc"]


def _host_prep(maskf, W_pvsd, b_pvsd, W_im):
    fblob = np.zeros((128, _FBLOB_W), dtype=np.float32)
    fblob[:, _ID0 : _ID0 + 128] = np.eye(128, dtype=np.float32)
    fblob[:, _W2LO : _W2LO + 64] = W_pvsd
    fblob[:, _W2HI + 64 : _W2HI + 128] = W_pvsd
    fblob[0:64, _BLO] = b_pvsd
    fblob[64:128, _BHI] = b_pvsd
    fblob[0:64, _WIMT : _WIMT + 64] = W_im.T
    fblob[:, _MASK : _MASK + L] = maskf
    fblob[:, _ONES] = 1.0
    tail = NROW % 128
    if tail:
        fblob[:tail, _TMASK] = 1.0
    else:
        fblob[:, _TMASK] = 1.0
    return fblob


def _core_inputs(c, nodes, sli, emb_i, fblob):
    b0 = c * BS
    flat = nodes[b0 : b0 + BS].reshape(-1)             # (800,) emb row ids
    gidx = np.zeros((128, NT), dtype=np.int32)
    for k in range(NT):
        seg = flat[k * 128 : (k + 1) * 128]
        gidx[: len(seg), k] = seg
    # local flat row r = b_local*L + l lands at (tile r//128, partition r%128)
    r = np.arange(NROW)
    p_of_r, k_of_r = r % 128, r // 128
    bglob = b0 + r // L
    selg = np.zeros((128, NT * 128), dtype=np.float32)  # [p, k*128+bglob] row-sum
    selg[p_of_r, k_of_r * 128 + bglob] = 1.0
    selt = np.zeros((128, NT * 128), dtype=np.float32)  # [bglob, k*128+p] Q broadcast
    selt[bglob, k_of_r * 128 + p_of_r] = 1.0
    sell = np.zeros((128, NT * 128), dtype=np.float32)  # last-click one-hots
    rl = (np.arange(BS) * L + sli[b0 : b0 + BS]).astype(np.int64)
    sell[rl % 128, (rl // 128) * 128 + (b0 + np.arange(BS))] = 1.0

    shard = emb_i[c * VS : (c + 1) * VS].T              # (64, VS)
    packed = np.ascontiguousarray(
        np.concatenate([shard[:, :HV], shard[:, HV:]], axis=0)
    )                                                   # (128, HV)
    return {
        "fblob": fblob,
        "gidx": gidx,
        "emb": emb_i,
        "embt": packed,
        "selg": selg,
        "selt": selt,
        "sell": sell,
    }


def kernel(
    nodes,
    categories,
    adj,
    nodes_categories,
    session_last_item_index,
    candidate_category,
    mask_node,
    emb_i,
    emb_c,
    Wd1,
    bd1,
    Wd2,
    bd2,
    W_pvsd,
    b_pvsd,
    W_im,
    _trace=False,
):
    nodes = np.ascontiguousarray(np.asarray(nodes, dtype=np.int32))
    sli = np.asarray(session_last_item_index).astype(np.int64)
    emb_i = np.ascontiguousarray(np.asarray(emb_i, dtype=np.float32))
    W_pvsd = np.asarray(W_pvsd, dtype=np.float32)
    b_pvsd = np.asarray(b_pvsd, dtype=np.float32)
    W_im = np.asarray(W_im, dtype=np.float32)
    maskf = np.asarray(mask_node, dtype=np.float32)

    fblob = _host_prep(maskf, W_pvsd, b_pvsd, W_im)
    in_maps = [_core_inputs(c, nodes, sli, emb_i, fblob) for c in range(N_CORES)]

    nc = _get_nc()
    res = run_bass_kernel_spmd(
        nc, in_maps, core_ids=list(range(N_CORES)), trace=_trace
    )
    _STATE["last_results"] = res

    P_v = np.concatenate([res.results[c]["pv"] for c in range(N_CORES)], axis=1)
    infomax_loss = np.asarray(res.results[0]["loss"][0, 0], dtype=np.float32)
    se = np.empty((B, L, D), dtype=np.float32)
    for c in range(N_CORES):
        se[c * BS : (c + 1) * BS] = res.results[c]["se_out"].reshape(BS, L, D)
    gnn = np.broadcast_to(se[:, None], (B, ND, L, D)).copy()
    return P_v, infomax_loss, gnn
